# revision 5
# baseline (speedup 1.0000x reference)
"""Anisotropic upsampling kernel for Trainium2 (8 NeuronCores, batch-sharded).

Computes, for inputs x0 (8,64,64,256), x1 (8,64,128,128), x2 (8,64,256,64):
    out0 = (up_h(x0) + up_w(x1)) / 2   -> (8,64,128,256)
    out1 = (up_h(x1) + up_w(x2)) / 2   -> (8,64,256,128)
where up() is the stride-2, length-5 normalized zero-insert upsampler:
    up(x)[2m]   = (x[m-1]+x[m]+x[m+1])/3   (edges: mean of the 2 valid taps)
    up(x)[2m+1] = (x[m]+x[m+1])/2          (edge m=W-1: x[W-1])

Layout: partitions = (h_half, c) -> p = ha*64 + c.  Every DRAM access is then
one fully contiguous run per partition (row ranges of one channel), which
keeps the DMA engines byte-bound instead of descriptor-bound.  Both up_h and
up_w are free-axis stencils: h-shifts are free-dim shifts by W, w-shifts by 1.
VectorE does the 2-/3-tap sums and the parity-interleaved accumulate adds;
ScalarE does the scaled copies (row-parity writes of the h-branch, prescales
of the w-branch).  All fp32.
"""

import numpy as np

_NC_CACHE = {}


def _build():
    import concourse.mybir as mybir
    from concourse import bacc
    from concourse.tile import TileContext

    f32 = mybir.dt.float32

    nc = bacc.Bacc("TRN2", target_bir_lowering=False, debug=False, num_devices=8)

    xs = {
        "x0": nc.dram_tensor("x0", [64, 64, 256], f32, kind="ExternalInput"),
        "x1": nc.dram_tensor("x1", [64, 128, 128], f32, kind="ExternalInput"),
        "x2": nc.dram_tensor("x2", [64, 256, 64], f32, kind="ExternalInput"),
    }
    out0 = nc.dram_tensor("out0", [64, 128, 256], f32, kind="ExternalOutput")
    out1 = nc.dram_tensor("out1", [64, 256, 128], f32, kind="ExternalOutput")

    with TileContext(nc) as tc:
        with (
            tc.tile_pool(name="inpool", bufs=3) as inpool,
            tc.tile_pool(name="stpool", bufs=2) as stpool,
            tc.tile_pool(name="opool", bufs=2) as opool,
        ):
            def do_output(out_d, xv_d, xh_d, H, W, R):
                """One output tensor: out = 0.5*up_h(xv) + 0.5*up_w(xh).

                out_d: (64, 2H, 2W) DRAM; xv_d: (64, H, 2W); xh_d: (64, 2H, W).
                Partition p = ha*64 + c covers output rows h2 = H*ha + r,
                r in [0, H).  R = output rows per chunk (R | H).
                """
                n_chunks = H // R
                for i in range(n_chunks):
                    r0 = i * R
                    # source rows of xv for this chunk: m in [m0, m0+R/2),
                    # with one halo row each side -> local rows j = m - (m0-1)
                    m0h = [H // 2 * ha + r0 // 2 for ha in range(2)]  # per half

                    XV = inpool.tile([128, R // 2 + 2, 2 * W], f32, tag="xv")
                    for ha in range(2):
                        lo = m0h[ha] - 1
                        hi = m0h[ha] + R // 2 + 1
                        jlo = 0
                        if lo < 0:
                            jlo, lo = 1, 0
                        if hi > H:
                            hi = H
                        nc.sync.dma_start(
                            out=XV[64 * ha:64 * (ha + 1), jlo:jlo + hi - lo, :],
                            in_=xv_d[:, lo:hi, :],
                        )
                    XH = inpool.tile([128, R, W], f32, tag="xh")
                    for ha in range(2):
                        nc.sync.dma_start(
                            out=XH[64 * ha:64 * (ha + 1)],
                            in_=xh_d[:, H * ha + r0:H * ha + r0 + R, :],
                        )

                    nh = R // 2  # source rows contributing this chunk
                    # ---- h-branch stencil sums (VectorE) ----
                    SH = stpool.tile([128, nh + 1, 2 * W], f32, tag="sh")
                    nc.vector.tensor_add(
                        SH, XV[:, 0:nh + 1, :], XV[:, 1:nh + 2, :])
                    TH = stpool.tile([128, nh, 2 * W], f32, tag="th")
                    nc.vector.tensor_add(
                        TH, SH[:, 0:nh, :], XV[:, 2:nh + 2, :])

                    # ---- w-branch stencil sums (VectorE) ----
                    SWE = stpool.tile([128, R, W], f32, tag="swe")
                    nc.vector.tensor_add(
                        SWE[:, :, 0:W - 1], XH[:, :, 0:W - 1], XH[:, :, 1:W])
                    TWE = stpool.tile([128, R, W], f32, tag="twe")
                    nc.vector.tensor_add(
                        TWE[:, :, 1:W - 1],
                        SWE[:, :, 0:W - 2], XH[:, :, 2:W])

                    # ---- h-branch scaled row-parity writes (ScalarE) ----
                    O = opool.tile([128, R, 2 * W], f32, tag="o")
                    nc.scalar.mul(O[:, 1:R:2, :], SH[:, 1:nh + 1, :], 0.25)
                    nc.scalar.mul(O[:, 0:R:2, :], TH, 1.0 / 6.0)

                    # ---- w-branch prescales into parity planes (ScalarE) ----
                    # SWE cols: 0..W-2 = 0.25*s_w, col W-1 = 0.5*x[W-1]
                    # TWE cols: 0 = 0.25*s_w[0], 1..W-2 = t_w/6,
                    #           col W-1 = 0.25*s_w[W-2]
                    nc.scalar.mul(
                        SWE[:, :, 0:W - 1], SWE[:, :, 0:W - 1], 0.25)
                    nc.scalar.mul(
                        SWE[:, :, W - 1:W], XH[:, :, W - 1:W], 0.5)
                    nc.scalar.mul(
                        TWE[:, :, 1:W - 1], TWE[:, :, 1:W - 1], 1.0 / 6.0)
                    nc.scalar.mul(
                        TWE[:, :, 0:W:W - 1],
                        SWE[:, :, 0:W - 1:W - 2], 1.0)

                    # ---- merge: O[parity cols] += plane (VectorE) ----
                    nc.vector.tensor_add(
                        O[:, :, 1:2 * W:2], O[:, :, 1:2 * W:2], SWE)
                    nc.vector.tensor_add(
                        O[:, :, 0:2 * W:2], O[:, :, 0:2 * W:2], TWE)

                    # ---- global h-edge fixups (half-partition ops) ----
                    if i == 0:
                        # h2 = 0 (even, m=0): 0.25*s_h[0] + w-plane
                        # s_h[0] = local SH row 1 on ha=0 partitions
                        nc.scalar.mul(O[0:64, 0, :], SH[0:64, 1, :], 0.25)
                        nc.vector.tensor_add(
                            O[0:64, 0, 1:2 * W:2],
                            O[0:64, 0, 1:2 * W:2], SWE[0:64, 0, :])
                        nc.vector.tensor_add(
                            O[0:64, 0, 0:2 * W:2],
                            O[0:64, 0, 0:2 * W:2], TWE[0:64, 0, :])
                    if i == n_chunks - 1:
                        # h2 = 2H-2 (even, m=H-1): 0.25*s_h[H-2] -> local
                        # SH row nh-1; h2 = 2H-1 (odd): 0.5*x[H-1] -> local
                        # XV row nh (ha=1 partitions)
                        nc.scalar.mul(
                            O[64:128, R - 2, :], SH[64:128, nh - 1, :], 0.25)
                        nc.scalar.mul(
                            O[64:128, R - 1, :], XV[64:128, nh, :], 0.5)
                        for rr in (R - 2, R - 1):
                            nc.vector.tensor_add(
                                O[64:128, rr, 1:2 * W:2],
                                O[64:128, rr, 1:2 * W:2], SWE[64:128, rr, :])
                            nc.vector.tensor_add(
                                O[64:128, rr, 0:2 * W:2],
                                O[64:128, rr, 0:2 * W:2], TWE[64:128, rr, :])

                    for ha in range(2):
                        nc.sync.dma_start(
                            out=out_d[:, H * ha + r0:H * ha + r0 + R, :],
                            in_=O[64 * ha:64 * (ha + 1)],
                        )

            do_output(out0, xs["x0"], xs["x1"], H=64, W=128, R=16)
            do_output(out1, xs["x1"], xs["x2"], H=128, W=64, R=32)

    nc.compile()
    return nc


def _get_nc():
    if "nc" not in _NC_CACHE:
        _NC_CACHE["nc"] = _build()
    return _NC_CACHE["nc"]


def kernel(x0, x1, x2):
    from concourse.bass_utils import run_bass_kernel_spmd

    nc = _get_nc()
    in_maps = [
        {
            "x0": np.ascontiguousarray(x0[b]),
            "x1": np.ascontiguousarray(x1[b]),
            "x2": np.ascontiguousarray(x2[b]),
        }
        for b in range(8)
    ]
    res = run_bass_kernel_spmd(nc, in_maps, core_ids=list(range(8)))
    o0 = np.stack([res.results[b]["out0"] for b in range(8)])
    o1 = np.stack([res.results[b]["out1"] for b in range(8)])
    return o0, o1
